# revision 61
# baseline (speedup 1.0000x reference)
"""Multi-head causal self-attention (B=4, N=2048, D=1024, H=16) on 8 TRN2 cores.

Sharding: 8 cores = 4 batches x 2 head-groups (8 heads / 512 dims each).
Per core (batch b, group g):
  - QKV projections computed in transposed layout (dims on partitions):
      Q^T, K^T = W^T-chunks (lhsT) x x^T (rhs), accumulated over 8 din chunks.
      V computed in natural [token, dv] layout (lhsT = x^T chunk).
  - Attention computed as S^T tiles [keys(128) x queries(512)] so that
    exp(S) feeds the P^T.V matmul directly (contraction over keys on
    partitions, no transposes anywhere). Softmax denominators come from a
    ones-column appended to V (row HD of the PV accumulator); normalization
    is deferred and batched per strip. Causal masking = skip blocks above
    the diagonal + multiply diagonal-region tiles by precomputed 0/1 masks
    after exp. No max-subtraction: scores are ~N(0,1) after the 1/sqrt(hd)
    scale, exp is safe in fp32.
  - O-projection partial: attnT (lhsT) x Wo-slice (rhs) -> [2048, 1024]
    partial output per core; host sums the two group partials per batch.

Dtypes: scores path float32r (fp32 storage, ~tf32 matmul precision, full PE
rate); P/V path bfloat16 (probs in [0,1], V ~N(0,1)).
"""

import numpy as np
import ml_dtypes

import concourse.bass as bass
import concourse.tile as tile
from concourse import bacc, mybir
from concourse import bass_utils
from concourse._compat import with_exitstack
from concourse.bass import ts, ds

B, N, D, H, HD = 4, 2048, 1024, 16, 64
GROUPS = 2              # head groups (cores per batch)
DC = D // GROUPS        # 512 dims per core
HPC = H // GROUPS       # 8 heads per core
P = 128
QW = 512                # query strip width / matmul free dim
NDIN = D // P           # 8 contraction chunks for QKV
NSTRIP = DC // P        # 4 dq strips per core (2 heads each)
NTT = N // P            # 16 token tiles
NTS = N // QW           # 4 token strips
NQB = QW // P           # 4 query blocks per strip

F32 = mybir.dt.float32
F32R = mybir.dt.float32r
BF16 = mybir.dt.bfloat16


def _emit(ctx, tc, xT, wq, wk, wv, wo, bq, bk, bv, masks, out):
    nc = tc.nc
    EXP = mybir.ActivationFunctionType.Exp

    const = ctx.enter_context(tc.tile_pool(name="const", bufs=1))
    p_mm = ctx.enter_context(tc.tile_pool(name="p_mm", bufs=2, space="PSUM"))
    p_pt = ctx.enter_context(tc.tile_pool(name="p_pt", bufs=4))
    p_small = ctx.enter_context(tc.tile_pool(name="p_small", bufs=2))
    p_dram = ctx.enter_context(tc.tile_pool(name="p_dram", bufs=2, space="DRAM"))

    # constants on the GpSimd (SWDGE) queue so they don't serialize with the
    # x^T stream on the sync (HWDGE) queue. maskt = one triangular 0/1 tile.
    maskt = const.tile([P, P], BF16)
    nc.gpsimd.dma_start(out=maskt, in_=masks)
    bqt = const.tile([P, NSTRIP], F32)
    nc.gpsimd.dma_start(out=bqt, in_=bq.rearrange("(s p) -> p s", p=P))
    bkt = const.tile([P, NSTRIP], F32)
    nc.gpsimd.dma_start(out=bkt, in_=bk.rearrange("(s p) -> p s", p=P))
    bvb = const.tile([P, DC], F32)      # loaded after strip-0 criticals

    # persistent per-batch tensors
    attnT = const.tile([P, NSTRIP, N], BF16)                # normalized attn^T
    vplus = const.tile([P, NTT, HPC, HD + 1], BF16)         # V | ones column
    # memset on an f32r/bf16 matmul-input tile is invalid ISA; write the ones
    # column via a DVE copy from an f32 staging tile (a valid rounding producer)
    ones_f32 = const.tile([P, NTT * HPC], F32)
    nc.vector.memset(ones_f32, 1.0)
    nc.vector.tensor_copy(
        out=vplus[:, :, :, HD:HD + 1],
        in_=ones_f32.rearrange("p (a b) -> p a b", b=HPC).unsqueeze(3),
    )


    # Wo tile loaded after strip-0 weights (not needed until phase C); single
    # multi-chunk descriptor per tensor — each dma_start costs ~600ns of issue
    # time on its queue engine, so batching chunks into one AP matters. All
    # weight/x DRAM tensors arrive host-pre-tiled so every DMA reads 8KB
    # contiguous per partition (1KB segments halve effective DMA bandwidth).
    wot = const.tile([P, NSTRIP, D], BF16)

    with tc.tile_pool(name="p_xt", bufs=1) as p_xt:
        # x^T resident, 64KB/part total, ONE TILE PER STRIP so dependency
        # tracking can't couple strip-0 consumers to strip 1-3 DMAs.
        # Compute start is gated on strip-0 x + strip-0 weights + (shortly
        # after) Wv: those split across the Pool and Activation DMA queues;
        # the sync queue carries only strips 1-3 so its FIFO never delays
        # the strip-0 critical set.
        # measured queue behavior: sync(HWDGE) starts ~8.3us and moves 8KB
        # descriptors fastest, so the whole critical sequence rides it in
        # consumption order. Strip 0 is split into two tiles so the first
        # QK matmuls start after only weights + half a strip have landed.
        xts = [p_xt.tile([P, NDIN // 2, QW], BF16, name="xts0a"),
               p_xt.tile([P, NDIN // 2, QW], BF16, name="xts0b")]
        xts += [p_xt.tile([P, NDIN, QW], BF16, name=f"xts{t}")
                for t in range(1, NTS)]

        def xtile(t, c):
            if t == 0:
                return xts[c // 4][:, c % 4, :]
            return xts[t + 1][:, c, :]

        with (
            tc.tile_pool(name="p_w", bufs=2) as p_w,
            tc.tile_pool(name="p_wv", bufs=1) as p_wv,
            tc.tile_pool(name="p_qk", bufs=2) as p_qk,
            tc.tile_pool(name="p_st", bufs=2, space="PSUM") as p_st,
            tc.tile_pool(name="p_pv", bufs=2, space="PSUM") as p_pv,
        ):
            # the sync queue moves ~4x the bytes/ns of the scalar/gpsimd
            # queues, so the WHOLE critical sequence rides it in exact
            # consumption order: x0, w0, x1, Wv, x2, x3. Only deferrable
            # loads (bv, strip>=1 weights, Wo) go elsewhere.
            wqs0 = p_w.tile([P, NDIN, P], BF16, tag="wq")
            wks0 = p_w.tile([P, NDIN, P], BF16, tag="wk")
            nc.sync.dma_start(out=wqs0, in_=wq[0])
            nc.sync.dma_start(out=xts[0], in_=xT[:, 0, 0:4])
            nc.sync.dma_start(out=wks0, in_=wk[0])
            nc.sync.dma_start(out=xts[1], in_=xT[:, 0, 4:8])
            nc.sync.dma_start(out=xts[2], in_=xT[:, 1])
            wvt = p_wv.tile([P, NDIN, DC], BF16)
            nc.sync.dma_start(out=wvt, in_=wv)
            nc.sync.dma_start(out=xts[3], in_=xT[:, 2])
            nc.sync.dma_start(out=xts[4], in_=xT[:, 3])
            nc.gpsimd.dma_start(
                out=bvb, in_=bv.unsqueeze(0).partition_broadcast(P))
            def attn_group(s, h2, qs, qts, kts, sums_sb):
                """S^T/exp/PV for one (head, query strip).

                Work units: full-width kc pairs below the diagonal region,
                then two packed diagonal units with shrinking query widths
                (512+384 and 256+128) — queries before the key block are
                skipped entirely, the remaining 128-wide leading wedge of
                each unit gets the triangular mask.
                """
                po = h2 * HD
                h = 2 * s + h2
                nfull = NQB * qs             # unmasked key blocks 0..nfull-1
                nkc = nfull + NQB
                q0 = qs * QW
                pvp = p_pv.tile([HD + 1, QW], F32, tag="pv", name="pvp")

                units = []
                for ip in range(nfull // 2):
                    units.append(("full", ip))
                units.append(("diagA", None))
                units.append(("diagB", None))

                def emit_s(unit):
                    kind, ip = unit
                    if kind == "full":
                        pst = p_st.tile([P, 2, QW], F32, tag="st", name="pst")
                        for j2 in range(2):
                            kc = 2 * ip + j2
                            nc.tensor.matmul(
                                pst[:, j2, :],
                                lhsT=kts[po:po + HD, ts(kc, P)],
                                rhs=qts[po:po + HD, ts(qs, QW)],
                                start=True, stop=True,
                            )
                        pt = p_pt.tile([P, 2, QW], BF16, tag="pt", name="pt")
                        nc.scalar.activation(out=pt, in_=pst, func=EXP, scale=0.125)
                        return pt
                    if kind == "diagA":
                        # j=0: kc=nfull,   queries [0:512), tri on cols 0:128
                        # j=1: kc=nfull+1, queries [128:512), tri on cols 0:128
                        pst = p_st.tile([P, 2, QW], F32, tag="st", name="pst")
                        nc.tensor.matmul(
                            pst[:, 0, :],
                            lhsT=kts[po:po + HD, ts(nfull, P)],
                            rhs=qts[po:po + HD, ts(qs, QW)],
                            start=True, stop=True,
                        )
                        nc.tensor.matmul(
                            pst[:, 1, 0:3 * P],
                            lhsT=kts[po:po + HD, ts(nfull + 1, P)],
                            rhs=qts[po:po + HD, ds(q0 + P, 3 * P)],
                            start=True, stop=True,
                        )
                        pt = p_pt.tile([P, 2, QW], BF16, tag="pt", name="pt")
                        # exp only the 896 live columns (cols 896:1024 of the
                        # flat tile are never consumed by PV)
                        nc.scalar.activation(
                            out=pt.rearrange("p a b -> p (a b)")[:, 0:896],
                            in_=pst.rearrange("p a b -> p (a b)")[:, 0:896],
                            func=EXP, scale=0.125)
                        nc.vector.tensor_mul(pt[:, 0, 0:P], pt[:, 0, 0:P], maskt)
                        nc.vector.tensor_mul(pt[:, 1, 0:P], pt[:, 1, 0:P], maskt)
                        return pt
                    # diagB: j=2: kc=nfull+2, queries [256:512) at cols 0:256;
                    #        j=3: kc=nfull+3, queries [384:512) at cols 256:384
                    pst = p_st.tile([P, QW], F32, tag="st", name="pst")
                    nc.tensor.matmul(
                        pst[:, 0:2 * P],
                        lhsT=kts[po:po + HD, ts(nfull + 2, P)],
                        rhs=qts[po:po + HD, ds(q0 + 2 * P, 2 * P)],
                        start=True, stop=True,
                    )
                    nc.tensor.matmul(
                        pst[:, 2 * P:3 * P],
                        lhsT=kts[po:po + HD, ts(nfull + 3, P)],
                        rhs=qts[po:po + HD, ds(q0 + 3 * P, P)],
                        start=True, stop=True,
                    )
                    pt = p_pt.tile([P, QW], BF16, tag="pt", name="pt")
                    # live columns are 0:384 only
                    nc.scalar.activation(out=pt[:, 0:3 * P], in_=pst[:, 0:3 * P],
                                         func=EXP, scale=0.125)
                    nc.vector.tensor_mul(pt[:, 0:P], pt[:, 0:P], maskt)
                    nc.vector.tensor_mul(pt[:, 2 * P:3 * P], pt[:, 2 * P:3 * P], maskt)
                    return pt

                def emit_pv(unit, pt):
                    kind, ip = unit
                    if kind == "full":
                        for j2 in range(2):
                            kc = 2 * ip + j2
                            nc.tensor.matmul(
                                pvp, lhsT=vplus[:, kc, h, :], rhs=pt[:, j2, :],
                                start=(kc == 0), stop=False,
                            )
                    elif kind == "diagA":
                        nc.tensor.matmul(
                            pvp, lhsT=vplus[:, nfull, h, :], rhs=pt[:, 0, :],
                            start=(nfull == 0), stop=False,
                        )
                        nc.tensor.matmul(
                            pvp[:, P:4 * P], lhsT=vplus[:, nfull + 1, h, :],
                            rhs=pt[:, 1, 0:3 * P], start=False, stop=False,
                        )
                    else:
                        nc.tensor.matmul(
                            pvp[:, 2 * P:4 * P], lhsT=vplus[:, nfull + 2, h, :],
                            rhs=pt[:, 0:2 * P], start=False, stop=False,
                        )
                        nc.tensor.matmul(
                            pvp[:, 3 * P:4 * P], lhsT=vplus[:, nfull + 3, h, :],
                            rhs=pt[:, 2 * P:3 * P], start=False, stop=True,
                        )

                LOOKP = 1
                pts = {}
                for i in range(len(units) + LOOKP):
                    if i < len(units):
                        pts[i] = emit_s(units[i])
                    if i >= LOOKP:
                        j = i - LOOKP
                        emit_pv(units[j], pts.pop(j))
                nc.vector.tensor_copy(
                    out=sums_sb[32 * qs:32 * qs + 1, h2, :],
                    in_=pvp[HD:HD + 1, :])
                nc.vector.tensor_copy(
                    out=attnT[po:po + HD, s, ts(qs, QW)], in_=pvp[0:HD, :])

            def normalize_h2(s, h2, sums_sb):
                """Batched softmax normalization for one head (4 query strips).

                1/s via the fast custom-DVE reciprocal (no activation-table
                loads — the Ln/Exp route reloads the Scalar engine's table
                twice per head at ~1.3us each and head-of-line blocks the
                next head's exp). Normalize multiplies run on GpSimd to keep
                Vector free for the inner-loop copies.
                """
                po = h2 * HD
                recip_sb = p_small.tile([P, QW], F32, tag="recip", name="recip_sb")
                nc.vector.reciprocal_approx_fast(
                    out=recip_sb, in_=sums_sb[:, h2, :])
                recb_sb = p_small.tile([P, QW], BF16, tag="recb", name="recb_sb")
                nc.vector.tensor_copy(out=recb_sb, in_=recip_sb)
                # broadcast across partitions via a DRAM round-trip
                # (SBUF-source partition-broadcast DMA is rejected); bf16
                # halves the 1MB/head broadcast traffic
                recip_d = p_dram.tile([NTS, QW], BF16, tag="recipd", name="recip_d")
                nc.sync.dma_start(
                    out=recip_d,
                    in_=recb_sb.rearrange("(a b) f -> a b f", b=32)[:, 0, :])
                # full-128-partition broadcast so rb[po:po+HD] shares the
                # base partition with the attnT slice (DVE rule); all 4
                # query strips in one issue
                rb = p_small.tile([P, NTS, QW], BF16, tag="rb", bufs=2, name="rb")
                nc.sync.dma_start(
                    out=rb, in_=recip_d.unsqueeze(0).partition_broadcast(P))
                for qs in range(NTS):
                    sl = attnT[po:po + HD, s, ts(qs, QW)]
                    nc.gpsimd.tensor_mul(
                        out=sl, in0=sl, in1=rb[po:po + HD, qs, :])

            def normalize_fast_qs(s, h2, qs, sums_sb):
                """Per-query-strip normalize for the LAST head: small
                per-strip DRAM round-trip broadcast + DVE mul, emitted with
                one-unit lookbehind, so phase C isn't gated on the batched
                multi-us store/load/Pool chain after the final PV."""
                po = h2 * HD
                r = 32 * qs
                # full-width recip (1-partition custom-DVE slices misbehave);
                # rows beyond the 4 real sums rows are memset 1.0 -> finite
                recq = p_small.tile([P, QW], F32, tag="recq", name="recq")
                nc.vector.reciprocal_approx_fast(
                    out=recq, in_=sums_sb[:, h2, :])
                recqd = p_dram.tile([1, QW], F32, tag="recqd", name="recqd")
                nc.sync.dma_start(out=recqd, in_=recq[r:r + 1, :])
                rbq = p_small.tile([P, QW], F32, tag="rbq", name="rbq")
                nc.sync.dma_start(
                    out=rbq,
                    in_=recqd[0, :].unsqueeze(0).partition_broadcast(P))
                sl = attnT[po:po + HD, s, ts(qs, QW)]
                nc.vector.tensor_mul(
                    out=sl, in0=sl, in1=rbq[po:po + HD, :])

            pending = [None]
            for s in range(NSTRIP):
                if s == 0:
                    wqs, wks = wqs0, wks0
                else:
                    wqs = p_w.tile([P, NDIN, P], BF16, tag="wq")
                    wks = p_w.tile([P, NDIN, P], BF16, tag="wk")
                    nc.gpsimd.dma_start(out=wqs, in_=wq[s])
                    nc.gpsimd.dma_start(out=wks, in_=wk[s])
                if s == 1:
                    nc.gpsimd.dma_start(out=wot, in_=wo)
                qts = p_qk.tile([P, N], BF16, tag="qt")
                kts = p_qk.tile([P, N], BF16, tag="kt")
                # sums rows at partition offsets {0,32,64,96} x 2 col blocks
                # (DVE partition offsets must be 32-aligned); unused rows are
                # memset to 1.0 so the batched reciprocal stays finite
                sums_sb = p_small.tile([P, 2, QW], F32, tag="sums")
                nc.gpsimd.memset(sums_sb, 1.0)
                def emit_qk(t):
                    psq = p_mm.tile([P, QW], F32, tag="mm", name="psq")
                    for c in range(NDIN):
                        nc.tensor.matmul(
                            psq, lhsT=wqs[:, c, :], rhs=xtile(t, c),
                            start=(c == 0), stop=(c == NDIN - 1),
                        )
                    # eviction via Scalar (idle during projection blocks;
                    # Vector's queue is backed up with attention mask-muls)
                    nc.scalar.activation(
                        out=qts[:, ts(t, QW)], in_=psq,
                        func=mybir.ActivationFunctionType.Identity,
                        bias=bqt[:, s:s + 1])
                    psk = p_mm.tile([P, QW], F32, tag="mm", name="psk")
                    for c in range(NDIN):
                        nc.tensor.matmul(
                            psk, lhsT=wks[:, c, :], rhs=xtile(t, c),
                            start=(c == 0), stop=(c == NDIN - 1),
                        )
                    nc.scalar.activation(
                        out=kts[:, ts(t, QW)], in_=psk,
                        func=mybir.ActivationFunctionType.Identity,
                        bias=bkt[:, s:s + 1])

                def emit_v(t):
                    # V = x @ Wv + bv, one token strip at a time right
                    # before the attention group that first needs it
                    for tt in range(NQB * t, NQB * (t + 1)):
                        psv = p_mm.tile([P, DC], F32, tag="mm", name="psv")
                        for c in range(NDIN):
                            nc.tensor.matmul(
                                psv,
                                lhsT=xtile(t, c)[:, ts(tt % NQB, P)],
                                rhs=wvt[:, c, :],
                                start=(c == 0), stop=(c == NDIN - 1),
                            )
                        nc.vector.tensor_add(
                            out=vplus[:, tt, :, 0:HD],
                            in0=psv.rearrange("p (h d) -> p h d", d=HD),
                            in1=bvb.rearrange("p (h d) -> p h d", d=HD),
                        )

                if s == 0:
                    # software-pipelined strip 0: QK(t+1) runs between QK(t)
                    # and V(t) so the chunk-sliced Wv load has time to land
                    emit_qk(0)
                    for t in range(NTS):
                        if t + 1 < NTS:
                            emit_qk(t + 1)
                        emit_v(t)
                        attn_group(s, 0, t, qts, kts, sums_sb)
                        attn_group(s, 1, t, qts, kts, sums_sb)
                        if t == 1 and pending[0] is not None:
                            pending[0](); pending[0] = None
                else:
                    for t in range(NTS):
                        emit_qk(t)
                if s == 0:
                    pending[0] = (lambda ss=s, sb=sums_sb:
                                  (normalize_h2(ss, 0, sb),
                                   normalize_h2(ss, 1, sb)))
                else:
                    # head-major; each head's normalization is emitted after
                    # the NEXT head's first groups so it backfills engine idle
                    # slots instead of head-of-line blocking the critical chain
                    for h2 in range(2):
                        last_head = (s == NSTRIP - 1 and h2 == 1)
                        for qs in range(NTS):
                            attn_group(s, h2, qs, qts, kts, sums_sb)
                            if qs == 1 and pending[0] is not None:
                                pending[0](); pending[0] = None
                            if last_head and qs >= 1:
                                normalize_fast_qs(s, h2, qs - 1, sums_sb)
                        if last_head:
                            normalize_fast_qs(s, h2, NTS - 1, sums_sb)
                        else:
                            pending[0] = (lambda ss=s, hh=h2, sb=sums_sb:
                                          normalize_h2(ss, hh, sb))

            if pending[0] is not None:
                pending[0](); pending[0] = None

    # ---- phase C: partial output = attnT^T @ Wo_slice ----
    with (
        tc.tile_pool(name="p_osb", bufs=3) as p_osb,
        tc.tile_pool(name="p_c", bufs=4, space="PSUM") as p_c,
    ):
        # software pipeline: emit chunks 0..2 of group g before the last
        # chunk + eviction of group g-LOOKC, so the PE has runway while the
        # final strip's normalization lands
        LOOKC = 3
        groups = [(tt, half) for tt in range(NTT) for half in range(2)]
        psos = {}
        osbs = {}
        for g in range(len(groups) + LOOKC):
            if g < len(groups):
                tt, half = groups[g]
                pso = p_c.tile([P, QW], F32, tag="c", name="pso")
                for c in range(NSTRIP - 1):
                    nc.tensor.matmul(
                        pso, lhsT=attnT[:, c, ts(tt, P)],
                        rhs=wot[:, c, ds(half * QW, QW)],
                        start=(c == 0), stop=False,
                    )
                psos[g] = pso
            if g >= LOOKC:
                tt, half = groups[g - LOOKC]
                pso = psos.pop(g - LOOKC)
                c = NSTRIP - 1
                nc.tensor.matmul(
                    pso, lhsT=attnT[:, c, ts(tt, P)],
                    rhs=wot[:, c, ds(half * QW, QW)],
                    start=False, stop=True,
                )
                if half == 0:
                    osbs[tt] = p_osb.tile([P, D], BF16, tag="osb", name="osb")
                nc.vector.tensor_copy(
                    out=osbs[tt][:, ds(half * QW, QW)], in_=pso)
                # bf16 out + store each half immediately: halves the store
                # bytes and drains the last strip right after its copy
                nc.sync.dma_start(
                    out=out[ts(tt, P), ds(half * QW, QW)],
                    in_=osbs[tt][:, ds(half * QW, QW)])
                if half == 1:
                    osbs.pop(tt)


_emit_wrapped = with_exitstack(_emit)

_NC_CACHE = None


def _build():
    global _NC_CACHE
    if _NC_CACHE is not None:
        return _NC_CACHE
    nc = bacc.Bacc("TRN2", target_bir_lowering=False, debug=False)
    # all inputs host-pre-tiled to the SBUF tile layouts (contiguous
    # per-partition runs -> minimal DMA descriptors)
    xT = nc.dram_tensor(
        "xt", [P, NTS, NDIN, QW], BF16, kind="ExternalInput").ap()
    wq = nc.dram_tensor(
        "wq", [NSTRIP, P, NDIN, P], BF16, kind="ExternalInput").ap()
    wk = nc.dram_tensor(
        "wk", [NSTRIP, P, NDIN, P], BF16, kind="ExternalInput").ap()
    wv = nc.dram_tensor(
        "wv", [P, NDIN, DC], BF16, kind="ExternalInput").ap()
    wo = nc.dram_tensor(
        "wo", [P, NSTRIP, D], BF16, kind="ExternalInput").ap()
    bq = nc.dram_tensor("bq", [DC], F32, kind="ExternalInput").ap()
    bk = nc.dram_tensor("bk", [DC], F32, kind="ExternalInput").ap()
    bv = nc.dram_tensor("bv", [DC], F32, kind="ExternalInput").ap()
    masks = nc.dram_tensor("masks", [P, P], BF16, kind="ExternalInput").ap()
    out = nc.dram_tensor("out", [N, D], BF16, kind="ExternalOutput").ap()
    with tile.TileContext(nc) as tc:
        _emit_wrapped(tc, xT, wq, wk, wv, wo, bq, bk, bv, masks, out)
    nc.compile()
    _NC_CACHE = nc
    return nc


def _make_masks():
    # triangular 0/1 tile for the diagonal blocks of S^T: key <= query kept
    return np.triu(np.ones((P, P), np.float32)).astype(ml_dtypes.bfloat16)


def _in_maps(x, Wq, bq, Wk, bk, Wv, bv, Wo):
    masks = _make_masks()
    bf = ml_dtypes.bfloat16
    maps = []
    # pre-tile into the exact SBUF layouts (see _build dram shapes)
    wq_g, wk_g, wv_g, wo_g = [], [], [], []
    for g in range(GROUPS):
        sl = slice(g * DC, (g + 1) * DC)
        wq_g.append(np.ascontiguousarray(
            Wq[:, sl].reshape(NDIN, P, NSTRIP, P).transpose(2, 1, 0, 3)
        ).astype(bf))
        wk_g.append(np.ascontiguousarray(
            Wk[:, sl].reshape(NDIN, P, NSTRIP, P).transpose(2, 1, 0, 3)
        ).astype(bf))
        wv_g.append(np.ascontiguousarray(
            Wv[:, sl].reshape(NDIN, P, DC).transpose(1, 0, 2)).astype(bf))
        wo_g.append(np.ascontiguousarray(
            Wo[sl, :].reshape(NSTRIP, P, D).transpose(1, 0, 2)).astype(bf))
    for b in range(B):
        xt_b = np.ascontiguousarray(
            np.asarray(x[b]).T.reshape(NDIN, P, NTS, QW).transpose(1, 2, 0, 3)
        ).astype(bf)
        for g in range(GROUPS):
            sl = slice(g * DC, (g + 1) * DC)
            maps.append({
                "xt": xt_b,
                "wq": wq_g[g],
                "wk": wk_g[g],
                "wv": wv_g[g],
                "wo": wo_g[g],
                "bq": np.ascontiguousarray(bq[sl]),
                "bk": np.ascontiguousarray(bk[sl]),
                "bv": np.ascontiguousarray(bv[sl]),
                "masks": masks,
            })
    return maps


def run(inputs, trace=False, tmpdir=None):
    """Build+run on 8 cores. Returns (out [B,N,D] f32, BassKernelResults)."""
    x = np.asarray(inputs["x"], np.float32)
    args = [np.asarray(inputs[k], np.float32) for k in
            ("Wq", "bq", "Wk", "bk", "Wv", "bv", "Wo")]
    bo = np.asarray(inputs["bo"], np.float32)
    nc = _build()
    maps = _in_maps(x, *args)
    if trace:
        bass_utils.upload_artifacts = lambda d: d
    res = bass_utils.run_bass_kernel_spmd(
        nc, maps, core_ids=list(range(8)), trace=trace, tmpdir=tmpdir)
    out = np.empty((B, N, D), np.float32)
    for b in range(B):
        out[b] = (res.results[2 * b]["out"].astype(np.float32)
                  + res.results[2 * b + 1]["out"].astype(np.float32) + bo)
    return out, res


def kernel(**inputs):
    out, _ = run(inputs)
    return out

